# revision 30
# baseline (speedup 1.0000x reference)
"""Trainium2 Bass kernel for nn_ClsTransformer (sparse kNN attention encoder).

Contract: kernel(**inputs) takes FULL unsharded inputs (x [8,1024,128] plus
stacked per-layer weights), shards batch across 8 NeuronCores (one batch
element per core, weights replicated), runs a fully fused per-core Bass/Tile
program, and returns the FULL [8,1024,128] output.

Perf notes (cost model): matmul rate keys on the MOVING operand dtype —
f32 = 4 cyc/row, f32r/bf16 = 1 cyc/row (moving free-dim >= 256). All matmul
moving operands here are produced as f32r (rounding producer ops) or bf16,
so every 512-wide matmul runs at full rate. kNN top-30 selection runs on DVE
(max8/match_replace x4) reading the distance PSUM tile directly; the
additive -1e9 mask is materialized as a single is_equal -> bf16 op and added
to logits via a (-1e9-scaled identity) @ maskT matmul.
"""

import sys

sys.path.insert(0, "/opt/trn_rl_repo")

import numpy as np

import bass_rust
import concourse.bass as bass
import concourse.mybir as mybir
from concourse.tile import TileContext
from concourse.masks import make_identity
from concourse.bass_utils import run_bass_kernel_spmd


def legalize_waits(nc, max_waits=1):
    """This container's walrus rejects instructions carrying more than ~1
    semaphore wait ("Too many sync wait commands"). Hoist extra waits onto
    preceding single-wait NoOps on the same engine (same stall semantics:
    the engine's sequencer blocks in order)."""
    n_split = 0
    for f in nc.m.functions:
        for bb in f.blocks:
            out = []
            for inst in bb.instructions:
                si = inst.sync_info
                if si is not None and len(si.on_wait) > max_waits:
                    waits = list(si.on_wait)
                    for k, w in enumerate(waits[:-max_waits]):
                        nop = mybir.InstNoOp(name=f"{inst.name}-w{k}")
                        nop.engine = inst.engine
                        nop.sync_info = bass_rust.SyncInfo(on_wait=[w], on_update=[])
                        out.append(nop)
                        n_split += 1
                    si.on_wait = waits[-max_waits:]
                out.append(inst)
            bb.instructions[:] = out
    return n_split

# Problem constants (hardcoded per contract)
B, N, D = 8, 1024, 128
H, DK, DV = 8, 64, 64
MID = 2048
KNN = 30
L = 3
P = 128
NEG = -1.0e9
EPS = 1e-5
NCH = N // P          # 8 chunks of 128 rows
NC2 = N // 512        # 2 chunks of 512

F32 = mybir.dt.float32
F32R = mybir.dt.float32r
BF16 = mybir.dt.bfloat16
AF = mybir.ActivationFunctionType
OP = mybir.AluOpType


def build_nc(legalize=True):
    nc = bass.Bass(trn_type="TRN2")
    x = nc.dram_tensor("x", [N, D], F32, kind="ExternalInput")
    wq = nc.dram_tensor("WQ", [L, D, H * DK], F32, kind="ExternalInput")
    wk = nc.dram_tensor("WK", [L, D, H * DK], F32, kind="ExternalInput")
    wv = nc.dram_tensor("WV", [L, D, H * DV], F32, kind="ExternalInput")
    wo = nc.dram_tensor("WO", [L, H * DV, D], F32, kind="ExternalInput")
    ff1 = nc.dram_tensor("FF1", [L, D, MID], F32, kind="ExternalInput")
    ff2 = nc.dram_tensor("FF2", [L, MID, D], F32, kind="ExternalInput")
    out = nc.dram_tensor("out", [N, D], F32, kind="ExternalOutput")

    with TileContext(nc) as tc:
        with (
            tc.tile_pool(name="const", bufs=1) as const,
            tc.tile_pool(name="persist", bufs=1) as persist,
            tc.tile_pool(name="hpool", bufs=1) as hpool,
            tc.tile_pool(name="wraw", bufs=1) as wraw,
            tc.tile_pool(name="wcvt", bufs=1) as wcvt,
            tc.tile_pool(name="work", bufs=2) as work,
            tc.tile_pool(name="rows", bufs=1) as rows,
            tc.tile_pool(name="scratch", bufs=1) as scratch,
            tc.tile_pool(name="ffp", bufs=3) as ffp,
            tc.tile_pool(name="psA", bufs=1, space="PSUM") as psA,   # dist [128,1024] = 2 banks
            tc.tile_pool(name="psB", bufs=2, space="PSUM") as psB,   # [*,512] misc = 2 banks
            tc.tile_pool(name="psL", bufs=2, space="PSUM") as psL,   # logits/transpose [128,512] = 2 banks
            tc.tile_pool(name="psAcc", bufs=2, space="PSUM") as psAcc,  # AV accum [65,512] = 2 banks
        ):
            # ---- constants ----
            ident_bf = const.tile([P, P], BF16)
            make_identity(nc, ident_bf)
            ident_f32 = const.tile([P, P], F32)
            make_identity(nc, ident_f32)
            # -1e9-scaled identity: matmul(identNEG, eqT) adds -1e9 where
            # eqT==1 (i.e. NOT a kNN neighbor)
            ident_neg = const.tile([P, P], BF16)
            nc.gpsimd.memset(ident_neg, 0.0)
            nc.gpsimd.affine_select(
                out=ident_neg, in_=ident_neg,
                compare_op=OP.not_equal, fill=NEG,
                base=0, pattern=[[-1, P]], channel_multiplier=1)
            # f32r const tiles: memset can't write f32r (walrus ISA check),
            # so memset f32 staging then convert via a rounding copy
            cst = const.tile([P, 3], F32)
            nc.vector.memset(cst[:, 0:1], -1.0)
            nc.vector.memset(cst[:, 1:2], 1.0 / D)
            nc.vector.memset(cst[:, 2:3], 1.0)
            neg_col = const.tile([P, 1], F32R)
            nc.scalar.copy(neg_col[:], cst[:, 0:1])
            inv_col = const.tile([P, 1], F32R)
            nc.scalar.copy(inv_col[:], cst[:, 1:2])
            ones_f = const.tile([1, P], F32)
            nc.vector.memset(ones_f, 1.0)
            ones_row = const.tile([1, P], F32R)
            nc.scalar.copy(ones_row[:], ones_f[:])
            eps_row = const.tile([1, 1], F32)
            nc.vector.memset(eps_row, EPS)

            # ---- load x, transpose into hT [D, N] (f32r) ----
            hT = hpool.tile([P, N], F32R, tag="hT")
            for ch in range(NCH):
                xt = work.tile([P, P], F32, tag="xload")
                nc.sync.dma_start(xt[:], x[ch * P:(ch + 1) * P, :])
                pt = psB.tile([P, P], F32, tag="B")
                nc.tensor.transpose(pt[:], xt[:], ident_f32[:])
                nc.vector.tensor_copy(hT[:, ch * P:(ch + 1) * P], pt[:])

            for l in range(L):
                # ---- weights: DMA raw f32, convert to f32r (ACT/DVE) ----
                wq_r = wraw.tile([P, H * DK], F32, tag="wq")
                wk_r = wraw.tile([P, H * DK], F32, tag="wk")
                wv_r = wraw.tile([P, H * DV], F32, tag="wv")
                wo_r = wraw.tile([P, 4, P], F32, tag="wo")
                ff1_r = wraw.tile([P, MID], F32, tag="ff1")
                ff2_r = wraw.tile([P, MID // P, P], F32, tag="ff2")
                nc.sync.dma_start(wq_r[:], wq[l])
                nc.sync.dma_start(wk_r[:], wk[l])
                nc.sync.dma_start(wv_r[:], wv[l])
                nc.sync.dma_start(wo_r[:], wo[l].rearrange("(k p) d -> p k d", p=P))
                nc.sync.dma_start(ff1_r[:], ff1[l])
                nc.sync.dma_start(ff2_r[:], ff2[l].rearrange("(k p) d -> p k d", p=P))
                wq_t = wcvt.tile([P, H * DK], F32R, tag="wq")
                wk_t = wcvt.tile([P, H * DK], F32R, tag="wk")
                wv_t = wcvt.tile([P, H * DV], F32R, tag="wv")
                wo_t = wcvt.tile([P, 4, P], F32R, tag="wo")
                ff1_t = wcvt.tile([P, MID], F32R, tag="ff1")
                ff2_t = wcvt.tile([P, MID // P, P], F32R, tag="ff2")
                # Q/sqrt(dk) folded into the WQ convert (0.125 is exact)
                nc.scalar.mul(wq_t[:], wq_r[:], 1.0 / np.sqrt(DK))
                nc.scalar.copy(wk_t[:], wk_r[:])
                nc.vector.tensor_copy(wv_t[:], wv_r[:])
                nc.vector.tensor_copy(wo_t[:], wo_r[:])
                nc.scalar.copy(ff1_t[:], ff1_r[:])
                nc.vector.tensor_copy(ff2_t[:], ff2_r[:])

                # ---- negsq[c] = -sum_d h[c,d]^2  (row [1, N]) ----
                hsq = scratch.tile([P, N], F32R, tag="tmp32")
                nc.scalar.activation(hsq[:], hT[:], AF.Square)
                negsq = rows.tile([1, N], F32R, tag="negsq")
                for c2 in range(NC2):
                    pns = psB.tile([1, 512], F32, tag="B")
                    nc.tensor.matmul(pns[:], neg_col[:],
                                     hsq[:, c2 * 512:(c2 + 1) * 512],
                                     start=True, stop=True)
                    nc.vector.tensor_copy(negsq[:, c2 * 512:(c2 + 1) * 512], pns[:])

                # hT2 = 2*hT (rhs of the distance matmul)
                hT2 = scratch.tile([P, N], F32R, tag="hT2")
                nc.scalar.mul(hT2[:], hT[:], 2.0)

                # ---- QKV projections ----
                # QT/KT: [hd, n] layout as [128, 4, N]; Q pre-scaled via WQ
                QT = persist.tile([P, 4, N], F32R, tag="QT")
                KT = persist.tile([P, 4, N], F32R, tag="KT")
                for m in range(4):
                    for c2 in range(NC2):
                        pq = psB.tile([P, 512], F32, tag="B")
                        nc.tensor.matmul(pq[:], wq_t[:, m * P:(m + 1) * P],
                                         hT[:, c2 * 512:(c2 + 1) * 512],
                                         start=True, stop=True)
                        nc.scalar.copy(QT[:, m, c2 * 512:(c2 + 1) * 512], pq[:])
                        pk = psB.tile([P, 512], F32, tag="B")
                        nc.tensor.matmul(pk[:], wk_t[:, m * P:(m + 1) * P],
                                         hT[:, c2 * 512:(c2 + 1) * 512],
                                         start=True, stop=True)
                        nc.scalar.copy(KT[:, m, c2 * 512:(c2 + 1) * 512], pk[:])
                # V: [n, hd] layout with appended ones column: [128, NCH, H, DV+1]
                V = persist.tile([P, NCH, H, DV + 1], F32R, tag="V")
                nc.vector.tensor_copy(
                    V[:, :, :, DV:DV + 1],
                    cst[:, 2:3].to_broadcast([P, NCH, H, 1]))
                for ch in range(NCH):
                    pv = psB.tile([P, 512], F32, tag="B")
                    nc.tensor.matmul(pv[:], hT[:, ch * P:(ch + 1) * P],
                                     wv_t[:], start=True, stop=True)
                    nc.scalar.copy(
                        V[:, ch, :, 0:DV],
                        pv[:].rearrange("p (h e) -> p h e", h=H))

                # ---- distances + exact top-30; eqT[c,kc,r]=1 iff NOT selected
                # The kNN mask for query half rc only needs query chunks
                # ch in [4rc, 4rc+4): build mask chunks 0-3, run attention on
                # queries 0-511 while chunks 4-7 select on DVE, then the
                # second attention pass.
                eqT = persist.tile([P, NCH, N], BF16, tag="eqT")
                oT = persist.tile([P, 4, N], F32R, tag="oT")

                def mask_chunk(ch):
                    pnd = psA.tile([P, N], F32, tag="A", name=f"pnd{l}_{ch}")
                    for c2 in range(NC2):
                        sl = slice(c2 * 512, (c2 + 1) * 512)
                        nc.tensor.matmul(pnd[:, sl], hT[:, ch * P:(ch + 1) * P],
                                         hT2[:, sl], start=True, stop=False)
                        nc.tensor.matmul(pnd[:, sl], ones_row[:],
                                         negsq[:, sl], start=False, stop=True)
                    # free the PSUM tile fast: selection runs on an SBUF copy
                    ndc = work.tile([P, N], F32, tag="ndc", name=f"ndc{l}_{ch}")
                    nc.scalar.copy(ndc[:], pnd[:])
                    # top-30: 4 rounds of max8 + match_replace (last keeps 6)
                    wrk = work.tile([P, N], F32, tag="ndwork", name=f"wrk{l}_{ch}")
                    mx = rows.tile([P, 8], F32, tag="mx", name=f"mx{l}_{ch}")
                    for rnd in range(4):
                        src = ndc if rnd == 0 else wrk
                        nc.vector.max(mx[:], src[:])
                        if rnd == 3:
                            nc.vector.memset(mx[:, 6:8], NEG)
                        nc.vector.match_replace(out=wrk[:], in_to_replace=mx[:],
                                                in_values=src[:], imm_value=NEG)
                    # eq: 1 where NOT selected (value untouched), 0 where selected
                    eqc = work.tile([P, N], BF16, tag="eqc", name=f"eqc{l}_{ch}")
                    nc.vector.tensor_tensor(eqc[:], ndc[:], wrk[:], OP.is_equal)
                    # transpose eq chunk [r=128, c=1024] -> eqT[c, :, r-chunk]
                    for g in range(2):
                        pt = psL.tile([P, 512], BF16, tag="L", name=f"pt{l}_{ch}_{g}")
                        for j in range(4):
                            cs = g * 4 + j
                            nc.tensor.transpose(pt[:, j * P:(j + 1) * P],
                                                eqc[:, cs * P:(cs + 1) * P], ident_bf[:])
                        nc.vector.tensor_copy(
                            eqT[:, g * 4:(g + 1) * 4, ch * P:(ch + 1) * P],
                            pt[:].rearrange("p (j r) -> p j r", j=4))

                def attn_head(rc, h):
                    sl = slice(rc * 512, (rc + 1) * 512)
                    bp = (h % 2) * DK
                    qsl = QT[bp:bp + DK, h // 2, :]
                    ksl = KT[bp:bp + DK, h // 2, :]
                    po = psAcc.tile([DV + 1, 512], F32, tag="acc",
                                    name=f"po{l}_{rc}_{h}")
                    for cs in range(NCH):
                        pl = psL.tile([P, 512], F32, tag="L", name=f"pl{l}_{rc}_{h}_{cs}")
                        nc.tensor.matmul(pl[:], ksl[:, cs * P:(cs + 1) * P],
                                         qsl[:, sl], start=True, stop=False)
                        nc.tensor.matmul(pl[:], ident_neg[:], eqT[:, cs, sl],
                                         start=False, stop=True)
                        eT = work.tile([P, 512], F32R, tag="eT", name=f"eT{l}_{rc}_{h}_{cs}")
                        nc.scalar.activation(eT[:], pl[:], AF.Exp)
                        nc.tensor.matmul(po[:], V[:, cs, h, :], eT[:],
                                         start=(cs == 0), stop=(cs == NCH - 1),
                                         skip_group_check=True)
                    return po

                def attn_norm(rc, h, po):
                    sl = slice(rc * 512, (rc + 1) * 512)
                    bp = (h % 2) * DK
                    rS = rows.tile([1, 512], F32R, tag="rS", name=f"rS{l}_{rc}_{h}")
                    with nc.allow_low_precision(reason="f32r denom recip; 2e-2 gate"):
                        nc.vector.reciprocal(rS[:], po[DV:DV + 1, :])
                    prs = psB.tile([DV, 512], F32, tag="B", name=f"prs{l}_{rc}_{h}")
                    nc.tensor.matmul(prs[:], ones_row[:, 0:DV], rS[:],
                                     start=True, stop=True)
                    rsb = work.tile([DV, 512], F32, tag="rsb", name=f"rsb{l}_{rc}_{h}")
                    nc.scalar.copy(rsb[:], prs[:])
                    nc.vector.tensor_tensor(
                        oT[bp:bp + DV, h // 2, sl], po[0:DV, :], rsb[:], OP.mult)

                for ch in range(4):
                    mask_chunk(ch)
                # attention on queries 0-511 overlapping kNN for chunks 4-7
                # (DVE). Each head's normalization is deferred one head so
                # its prs matmul never head-of-line blocks PE on the recip.
                seq = [(0, h) for h in range(H)] + [(1, h) for h in range(H)]
                prev = None
                mask_next = 4
                for rc, h in seq:
                    po = attn_head(rc, h)
                    if prev is not None:
                        attn_norm(*prev)
                        if mask_next < NCH:
                            mask_chunk(mask_next)
                            mask_next += 1
                    prev = (rc, h, po)
                attn_norm(*prev)

                # ---- WO + residual + LN -> yT ----
                z1 = scratch.tile([P, N], F32R, tag="z1")
                for rc in range(NC2):
                    sl = slice(rc * 512, (rc + 1) * 512)
                    pz = psB.tile([P, 512], F32, tag="B")
                    for kt in range(4):
                        nc.tensor.matmul(pz[:], wo_t[:, kt, :], oT[:, kt, sl],
                                         start=(kt == 0), stop=(kt == 3))
                    nc.vector.scalar_tensor_tensor(z1[:, sl], pz[:], 1.0, hT[:, sl],
                                                   OP.mult, OP.add)
                yT = scratch.tile([P, N], F32R, tag="yT")
                layer_norm(nc, scratch, rows, psB, inv_col, ones_row, eps_row, z1, yT)

                # ---- FFN: relu(yT @ FF1) @ FF2 + residual + LN -> next hT ----
                z2 = scratch.tile([P, N], F32R, tag="z2")
                for rc in range(NC2):
                    sl = slice(rc * 512, (rc + 1) * 512)
                    pz2 = psB.tile([P, 512], F32, tag="B")
                    for kt in range(MID // P):
                        pf = psL.tile([P, 512], F32, tag="L")
                        nc.tensor.matmul(pf[:], ff1_t[:, kt * P:(kt + 1) * P],
                                         yT[:, sl], start=True, stop=True)
                        ffs = ffp.tile([P, 512], F32R, tag="ff")
                        if kt % 2 == 0:
                            nc.scalar.activation(ffs[:], pf[:], AF.Relu)
                        else:
                            nc.vector.tensor_scalar_max(ffs[:], pf[:], 0.0)
                        nc.tensor.matmul(pz2[:], ff2_t[:, kt, :], ffs[:],
                                         start=(kt == 0), stop=(kt == MID // P - 1),
                                         skip_group_check=True)
                    nc.vector.scalar_tensor_tensor(z2[:, sl], pz2[:], 1.0, yT[:, sl],
                                                   OP.mult, OP.add)
                hT = hpool.tile([P, N], F32R, tag="hT")
                layer_norm(nc, scratch, rows, psB, inv_col, ones_row, eps_row, z2, hT)

            # ---- output: transpose hT back to [N, D] ----
            for ch in range(NCH):
                pt = psB.tile([P, P], F32, tag="B")
                nc.tensor.transpose(pt[:], hT.bitcast(F32)[:, ch * P:(ch + 1) * P],
                                    ident_f32[:])
                ot = work.tile([P, P], F32, tag="xload")
                nc.vector.tensor_copy(ot[:], pt[:])
                nc.sync.dma_start(out[ch * P:(ch + 1) * P, :], ot[:])
    if legalize:
        legalize_waits(nc)
    return nc


def layer_norm(nc, scratch, rows, psB, inv_col, ones_row, eps_row, zT, outT):
    """outT = (zT - mean) * rsqrt(var + eps), stats over the partition (D) axis.

    mean/E[z^2] via ones-matmuls, rstd = exp(-0.5*log(var+eps)) on ACT
    (the Sqrt table is too coarse), broadcast back via K=1 outer-product MMs.
    """
    zsq = scratch.tile([P, N], F32R, tag="tmp32")
    nc.scalar.activation(zsq[:], zT[:], AF.Square)
    mean = rows.tile([1, N], F32R, tag="mean")
    msq = rows.tile([1, N], F32R, tag="msq")
    for c2 in range(NC2):
        sl = slice(c2 * 512, (c2 + 1) * 512)
        pm = psB.tile([1, 512], F32, tag="B")
        nc.tensor.matmul(pm[:], inv_col[:], zT[:, sl], start=True, stop=True)
        nc.vector.tensor_copy(mean[:, sl], pm[:])
        pm2 = psB.tile([1, 512], F32, tag="B")
        nc.tensor.matmul(pm2[:], inv_col[:], zsq[:, sl], start=True, stop=True)
        nc.vector.tensor_copy(msq[:, sl], pm2[:])
    rowtmp = rows.tile([1, N], F32R, tag="rowtmp")
    nc.vector.tensor_tensor(rowtmp[:], mean[:], mean[:], OP.mult)
    nc.vector.tensor_tensor(msq[:], msq[:], rowtmp[:], OP.subtract)   # msq := var
    nc.scalar.activation(rowtmp[:], msq[:], AF.Ln, bias=eps_row[:])   # rowtmp := ln(var+eps)
    nc.scalar.activation(msq[:], rowtmp[:], AF.Exp, scale=-0.5)       # msq := rstd
    rstd = msq
    brow = rowtmp
    nc.vector.scalar_tensor_tensor(brow[:], mean[:], -1.0, rstd[:], OP.mult, OP.mult)
    for c2 in range(NC2):
        sl = slice(c2 * 512, (c2 + 1) * 512)
        pa = psB.tile([P, 512], F32, tag="B")
        nc.tensor.matmul(pa[:], ones_row[:], rstd[:, sl], start=True, stop=True)
        pb = psB.tile([P, 512], F32, tag="B")
        nc.tensor.matmul(pb[:], ones_row[:], brow[:, sl], start=True, stop=True)
        nc.vector.tensor_tensor(outT[:, sl], zT[:, sl], pa[:], OP.mult)
        nc.vector.tensor_tensor(outT[:, sl], outT[:, sl], pb[:], OP.add)



_nc_cache = None


def kernel(**inputs):
    global _nc_cache
    if _nc_cache is None:
        _nc_cache = build_nc()
    nc = _nc_cache
    x = np.ascontiguousarray(inputs["x"], dtype=np.float32)
    shared = {k: np.ascontiguousarray(np.asarray(inputs[k]), dtype=np.float32)
              for k in ("WQ", "WK", "WV", "WO", "FF1", "FF2")}
    in_maps = [dict(x=x[b], **shared) for b in range(B)]
    res = run_bass_kernel_spmd(nc, in_maps, core_ids=list(range(B)))
    return np.stack([res.results[b]["out"] for b in range(B)], axis=0)


if __name__ == "__main__":
    nc = build_nc()
    print("built ok")
